# revision 11
# baseline (speedup 1.0000x reference)
"""Causal single-head attention (B=4, S=2048, D=1024) on 8 Trainium2 NeuronCores.

Sharding: core = (batch, parity). Each batch's 4 query-chunks of 512 are split
{0,3} / {1,2} across its two cores so causal work balances exactly.

Key trick: scores = q@k.T = x_q (Wq^T Wk) x_k^T. The host precomputes
A = Wq^T Wk / 32 exactly in fp32, so the device only needs ONE projection
(t = x_q A, query-sharded) for the score path instead of replicated Q and K
projections — the scores matmul contracts t directly against the raw x
tiles already resident in SBUF. This removes the entire K projection
(~55us of PE time per core).

All inputs are host-retiled so that every SBUF tile loads with one large
contiguous DMA ([p, i*free] packed layout: column block i of a tile carries
contraction rows 128i..128(i+1)); loads are spread over the three DGE rings
(sync + scalar HWDGE, gpsimd SWDGE) in deadline order: the first
v-projection group's operands (wv half-tiles + xs[0]) lead on all three
rings, the rest of xs streams at its consumption pace, and a/xq (needed
only when the t-projection starts ~60us in) trail.

Device algorithm per core (all matmuls bf16 operands, fp32 PSUM accumulation):
  v[s,o]  = x.T @ WvT
  tT[o,s] = A'.T @ xTq          (A' = Wq^T Wk / 32, host-exact)
  scores_T[sk,sq] = x_blk.T @ tT_chunk   -> PSUM
  p = exp(scores_T) * mask   (no max-subtraction: logits are O(1) by
                              construction; masked entries multiply to 0)
  out[sq,o] = sum_blk p_blk.T @ v_blk ; den[sq] = sum_blk p_blk.T @ ones
  out = out * (1/den)        (stored bf16 to halve output DMA)
"""

import sys

if "/opt/trn_rl_repo" not in sys.path:
    sys.path.insert(0, "/opt/trn_rl_repo")

import numpy as np
import ml_dtypes

import concourse.mybir as mybir
import concourse.tile as tile
from concourse import bacc
from concourse.bass_utils import run_bass_kernel_spmd

bf16 = ml_dtypes.bfloat16

B, S, D = 4, 2048, 1024
CH = 512            # projection column-chunk width
QC = 256            # attention query-chunk width
BLK = 128           # key-block
NST = S // BLK      # 16 sequence tiles
# Per-core schedule: 4 query-chunks of 256, processed with a fixed padded
# k-block count (4,8,12,16). Host assigns real chunks sorted by causal depth
# so padding waste is exactly 4 blocks/core; masks (data) encode reality.
SCHED = (4, 8, 12, 16)
MASK_BASE = (0, 4, 12, 24)
NMASK = sum(SCHED)  # 40
DT8 = D // 128      # contraction tiles
N_CORES = 8
DT_BF = mybir.dt.bfloat16
DT_F32 = mybir.dt.float32

_NC_CACHE = {}


def _emit(tc, xs, xq, aT, wv, msk, out):
    nc = tc.nc
    Exp = mybir.ActivationFunctionType.Exp

    with (
        tc.tile_pool(name="const", bufs=1) as constp,
        tc.tile_pool(name="kv", bufs=1) as kv,
    ):
        ones = constp.tile([128, 1], DT_BF, tag="ones", name="ones")
        nc.vector.memset(ones, 1.0)

        # x stays resident through attention: it doubles as the K operand
        # of the scores matmul (the A-trick) and feeds the V projection.
        # xs_t[st][:, 128i:128(i+1)] = x.T[128i:128(i+1), 128st:128(st+1)]
        xs_t = [kv.tile([128, D], DT_BF, tag=f"xs{st}", name=f"xs{st}")
                for st in range(NST)]
        v_t = [kv.tile([128, D], DT_BF, tag=f"v{st}", name=f"v{st}")
               for st in range(NST)]
        tT_t = [kv.tile([128, 2 * CH], DT_BF, tag=f"tT{i}", name=f"tT{i}")
                for i in range(DT8)]

        with (
            tc.tile_pool(name="xw", bufs=1) as xw,
            tc.tile_pool(name="proj_ps", bufs=2, space="PSUM") as pps,
        ):
            # wv_t[oc][:, 512i:512(i+1)] = Wv.T[128i:128(i+1), 512oc:...]
            wv_t = [xw.tile([128, DT8 * CH], DT_BF, tag=f"wv{oc}", name=f"wv{oc}")
                    for oc in range(D // CH)]
            # a_t[:, 1024i+c] = A'[128i+p, c];  xq_t[:, 1024i+c] = xTq[128i+p, c]
            a_t = xw.tile([128, DT8 * D], DT_BF, tag="a", name="a")
            xq_t = xw.tile([128, DT8 * D], DT_BF, tag="xq", name="xq")

            # Deadline-ordered loads: first group's gate (wv[0] + xs[0],
            # ~1.25MB) split across all three rings ahead of everything
            # else; xs streams at v-projection pace; wv[1] before the oc=1
            # pass (~35us); a/xq before the t-projection (~60us).
            # first-group gate rides the two HWDGE rings only — SWDGE
            # (gpsimd) has slow startup and gets mid-stream tiles.
            nc.sync.dma_start(out=xs_t[0], in_=xs[0])
            nc.scalar.dma_start(out=wv_t[0][:, 0 : 2 * CH], in_=wv[0][:, 0 : 2 * CH])
            nc.sync.dma_start(out=wv_t[0][:, 2 * CH :], in_=wv[0][:, 2 * CH :])
            nc.scalar.dma_start(out=xs_t[1], in_=xs[1])
            order = [(2, nc.gpsimd), (3, nc.scalar), (4, nc.sync),
                     (5, nc.gpsimd), (6, nc.scalar), (7, nc.sync),
                     (8, nc.gpsimd), (9, nc.scalar), (10, nc.sync)]
            for st, eng in order:
                eng.dma_start(out=xs_t[st], in_=xs[st])
            nc.scalar.dma_start(out=wv_t[1], in_=wv[1])
            for st, eng in [(11, nc.gpsimd), (12, nc.scalar), (13, nc.sync),
                            (14, nc.gpsimd), (15, nc.scalar)]:
                eng.dma_start(out=xs_t[st], in_=xs[st])
            nc.sync.dma_start(out=a_t[:, DT8 * D // 2 :], in_=aT[1])
            nc.scalar.dma_start(out=a_t[:, 0 : DT8 * D // 2], in_=aT[0])
            nc.gpsimd.dma_start(out=xq_t[:, 0 : DT8 * D // 2], in_=xq[0])
            nc.sync.dma_start(out=xq_t[:, DT8 * D // 2 :], in_=xq[1])

            # v projection, oc-outer so the first 16 groups need only wv[0]:
            # v[s,o] += x[i,s].T @ WvT[i,o]
            for oc in range(D // CH):
                for st in range(NST):
                    ps = pps.tile([128, CH], DT_F32, tag="pps", name="pps")
                    for i in range(DT8):
                        nc.tensor.matmul(
                            ps,
                            lhsT=xs_t[st][:, 128 * i : 128 * (i + 1)],
                            rhs=wv_t[oc][:, CH * i : CH * (i + 1)],
                            start=(i == 0),
                            stop=(i == DT8 - 1),
                        )
                    nc.scalar.copy(out=v_t[st][:, CH * oc : CH * (oc + 1)], in_=ps)
            # tT projection (score precursor; A' = Wq^T Wk / 32 on host):
            # tT[o,s] = A'[i,o].T @ xTq[i,s]
            for ot in range(DT8):
                for qc in range(2):
                    ps = pps.tile([128, CH], DT_F32, tag="pps", name="pps")
                    for i in range(DT8):
                        nc.tensor.matmul(
                            ps,
                            lhsT=a_t[:, D * i + 128 * ot : D * i + 128 * (ot + 1)],
                            rhs=xq_t[:, D * i + CH * qc : D * i + CH * (qc + 1)],
                            start=(i == 0),
                            stop=(i == DT8 - 1),
                        )
                    nc.scalar.copy(out=tT_t[ot][:, CH * qc : CH * (qc + 1)], in_=ps)

        # ---- attention ----
        with (
            tc.tile_pool(name="attn_sb", bufs=1) as asb,
            tc.tile_pool(name="mask_sb", bufs=4) as msb,
            tc.tile_pool(name="outs_sb", bufs=2) as osb,
            tc.tile_pool(name="score_ps", bufs=2, space="PSUM") as sps,
            tc.tile_pool(name="out_ps", bufs=2, space="PSUM") as ops,
            tc.tile_pool(name="den_ps", bufs=2, space="PSUM") as dps,
        ):
            p_t = {}
            for L in range(4):
                for b in range(SCHED[L]):
                    ps = sps.tile([128, QC], DT_F32, tag="sps", name="sps")
                    for i in range(DT8):
                        nc.tensor.matmul(
                            ps,
                            lhsT=xs_t[b][:, 128 * i : 128 * (i + 1)],
                            rhs=tT_t[i][:, QC * L : QC * (L + 1)],
                            start=(i == 0),
                            stop=(i == DT8 - 1),
                        )
                    m = msb.tile([128, QC], DT_BF, tag="mask", name="mask")
                    nc.sync.dma_start(out=m, in_=msk[MASK_BASE[L] + b])
                    es = msb.tile([128, QC], DT_BF, tag="es", name="es")
                    nc.scalar.activation(es, ps, Exp)
                    p = asb.tile([128, QC], DT_BF, tag=f"p{L}_{b}", name=f"p{L}_{b}")
                    nc.vector.tensor_mul(p, es, m)
                    p_t[(L, b)] = p

            for L in range(4):
                nblk = SCHED[L]
                for sqt in range(QC // 128):
                    # den first within each block, and the output halves in
                    # separate PSUM tiles: the reciprocal and the first
                    # half's scale+store start before the last matmuls of
                    # the second half finish, shortening the exposed tail.
                    po0 = ops.tile([128, CH], DT_F32, tag="po0", name="po0")
                    po1 = ops.tile([128, CH], DT_F32, tag="po1", name="po1")
                    pd = dps.tile([128, 1], DT_F32, tag="pd", name="pd")
                    for b in range(nblk):
                        pt = p_t[(L, b)][:, 128 * sqt : 128 * (sqt + 1)]
                        nc.tensor.matmul(
                            pd, lhsT=pt, rhs=ones,
                            start=(b == 0), stop=(b == nblk - 1),
                            skip_group_check=True,
                        )
                        nc.tensor.matmul(
                            po0, lhsT=pt, rhs=v_t[b][:, 0:CH],
                            start=(b == 0), stop=(b == nblk - 1),
                            skip_group_check=True,
                        )
                        nc.tensor.matmul(
                            po1, lhsT=pt, rhs=v_t[b][:, CH:D],
                            start=(b == 0), stop=(b == nblk - 1),
                            skip_group_check=True,
                        )
                    r = osb.tile([128, 1], DT_F32, tag="r", name="r")
                    nc.vector.reciprocal(r, pd)
                    o = osb.tile([128, D], DT_BF, tag="osb", name="osb")
                    row = QC * L + 128 * sqt
                    nc.vector.tensor_scalar_mul(o[:, 0:CH], po0, r)
                    nc.sync.dma_start(
                        out=out[row : row + 128, 0:CH], in_=o[:, 0:CH])
                    nc.vector.tensor_scalar_mul(o[:, CH:D], po1, r)
                    nc.scalar.dma_start(
                        out=out[row : row + 128, CH:D], in_=o[:, CH:D])


def build_program():
    nc = bacc.Bacc(
        "TRN2",
        target_bir_lowering=False,
        debug=False,
        enable_asserts=False,
        num_devices=N_CORES,
    )
    xs = nc.dram_tensor("xs", [NST, 128, D], DT_BF, kind="ExternalInput").ap()
    xq = nc.dram_tensor("xq", [2, 128, DT8 * D // 2], DT_BF, kind="ExternalInput").ap()
    aT = nc.dram_tensor("aT", [2, 128, DT8 * D // 2], DT_BF, kind="ExternalInput").ap()
    wv = nc.dram_tensor("wv", [D // CH, 128, DT8 * CH], DT_BF, kind="ExternalInput").ap()
    msk = nc.dram_tensor("msk", [NMASK, BLK, QC], DT_BF, kind="ExternalInput").ap()
    out = nc.dram_tensor("out", [2 * CH, D], DT_BF, kind="ExternalOutput").ap()
    with tile.TileContext(nc) as tc:
        _emit(tc, xs, xq, aT, wv, msk, out)
    nc.compile()
    return nc


def get_program():
    if "nc" not in _NC_CACHE:
        _NC_CACHE["nc"] = build_program()
    return _NC_CACHE["nc"]


def _chunks_for(core):
    """Per-core 256-wide query chunks, L-ordered to match SCHED=(4,8,12,16).
    Real causal k-block need: chunk j -> 2(j+1)."""
    return [0, 3, 4, 7] if core % 2 == 0 else [1, 2, 5, 6]


def _build_masks(chunks):
    """[40,128,256] in {0,1}: allowed(sk=128*blk+p, sq=256*j+c) = sk <= sq.
    Padding blocks beyond a chunk's real causal depth come out all-zero."""
    m = np.zeros((NMASK, BLK, QC), np.float32)
    p = np.arange(BLK)[:, None]
    c = np.arange(QC)[None, :]
    for L, j in enumerate(chunks):
        for b in range(SCHED[L]):
            m[MASK_BASE[L] + b] = BLK * b + p <= QC * j + c
    return m.astype(bf16)


def _pack_pi(mat, free):
    """[128*8, ncol] -> [2, 128, 4*ncol]: col block i carries rows 128i.."""
    r = mat.reshape(DT8, 128, -1).transpose(1, 0, 2).reshape(128, -1)
    return np.ascontiguousarray(r.reshape(128, 2, free).transpose(1, 0, 2))


def build_in_maps(x, Wq, Wk, Wv):
    # A' = Wq^T Wk / 32 (exact in fp32 on host): scores = x_q A' x_k^T.
    a = (Wq.T.astype(np.float32) @ Wk.astype(np.float32)) / 32.0
    a8 = _pack_pi(a.astype(bf16), DT8 * D // 2)
    # wv[oc][p][512i+c] = Wv[512oc+c, 128i+p]
    wv8 = np.ascontiguousarray(
        Wv.astype(bf16).reshape(D // CH, CH, DT8, 128).transpose(0, 3, 2, 1)
        .reshape(D // CH, 128, DT8 * CH))
    masks = {par: _build_masks(_chunks_for(par)) for par in (0, 1)}
    in_maps = []
    for core in range(N_CORES):
        b = core // 2
        chunks = _chunks_for(core)
        xb = x[b].astype(bf16)  # [S, D]
        # xs[st][p][128i+c] = x[128st+c, 128i+p]
        xs = np.ascontiguousarray(
            xb.reshape(NST, BLK, DT8, 128).transpose(0, 3, 2, 1)
            .reshape(NST, 128, D))
        xqc = np.concatenate(
            [xb[QC * j : QC * (j + 1)].T for j in chunks], axis=1)  # [D, 1024]
        in_maps.append(
            {"xs": xs, "xq": _pack_pi(xqc, DT8 * D // 2), "aT": a8,
             "wv": wv8, "msk": masks[core % 2]}
        )
    return in_maps


def assemble_output(results):
    out = np.zeros((B, S, D), np.float32)
    for core in range(N_CORES):
        b = core // 2
        for L, j in enumerate(_chunks_for(core)):
            out[b, QC * j : QC * (j + 1)] = (
                results[core]["out"][QC * L : QC * (L + 1)].astype(np.float32)
            )
    return out


def kernel(x, Wq, Wk, Wv):
    x = np.asarray(x, np.float32)
    nc = get_program()
    in_maps = build_in_maps(x, np.asarray(Wq, np.float32),
                            np.asarray(Wk, np.float32), np.asarray(Wv, np.float32))
    res = run_bass_kernel_spmd(nc, in_maps, core_ids=list(range(N_CORES)))
    return assemble_output(res.results)


# revision 12
# speedup vs baseline: 1.0659x; 1.0659x over previous
"""Causal single-head attention (B=4, S=2048, D=1024) on 8 Trainium2 NeuronCores.

Sharding: core = (batch, parity). Each batch's 4 query-chunks of 512 are split
{0,3} / {1,2} across its two cores so causal work balances exactly.

Key trick: scores = q@k.T = x_q (Wq^T Wk) x_k^T. The host precomputes
A = Wq^T Wk / 32 exactly in fp32, so the device only needs ONE projection
(t = x_q A, query-sharded) for the score path instead of replicated Q and K
projections — the scores matmul contracts t directly against the raw x
tiles already resident in SBUF. This removes the entire K projection
(~55us of PE time per core).

All inputs are host-retiled so that every SBUF tile loads with one large
contiguous DMA ([p, i*free] packed layout: column block i of a tile carries
contraction rows 128i..128(i+1)); loads are spread over the three DGE rings
(sync + scalar HWDGE, gpsimd SWDGE) in deadline order: the first
v-projection group's operands (wv half-tiles + xs[0]) lead on all three
rings, the rest of xs streams at its consumption pace, and a/xq (needed
only when the t-projection starts ~60us in) trail.

Device algorithm per core (all matmuls bf16 operands, fp32 PSUM accumulation):
  v[s,o]  = x.T @ WvT
  tT[o,s] = A'.T @ xTq          (A' = Wq^T Wk / 32, host-exact)
  scores_T[sk,sq] = x_blk.T @ tT_chunk   -> PSUM
  p = exp(scores_T) * mask   (no max-subtraction: logits are O(1) by
                              construction; masked entries multiply to 0)
  out[sq,o] = sum_blk p_blk.T @ v_blk ; den[sq] = sum_blk p_blk.T @ ones
  out = out * (1/den)        (stored bf16 to halve output DMA)
"""

import sys

if "/opt/trn_rl_repo" not in sys.path:
    sys.path.insert(0, "/opt/trn_rl_repo")

import numpy as np
import ml_dtypes

import concourse.mybir as mybir
import concourse.tile as tile
from concourse import bacc
from concourse.bass_utils import run_bass_kernel_spmd

bf16 = ml_dtypes.bfloat16

B, S, D = 4, 2048, 1024
CH = 512            # projection column-chunk width
QC = 256            # attention query-chunk width
BLK = 128           # key-block
NST = S // BLK      # 16 sequence tiles
# Per-core schedule: 4 query-chunks of 256, processed with a fixed padded
# k-block count (4,8,12,16). Host assigns real chunks sorted by causal depth
# so padding waste is exactly 4 blocks/core; masks (data) encode reality.
SCHED = (4, 8, 12, 16)
MASK_BASE = (0, 4, 12, 24)
NMASK = sum(SCHED)  # 40
DT8 = D // 128      # contraction tiles
N_CORES = 8
DT_BF = mybir.dt.bfloat16
DT_F32 = mybir.dt.float32

_NC_CACHE = {}


def _emit(tc, xs, xq, aT, wv, msk, out):
    nc = tc.nc
    Exp = mybir.ActivationFunctionType.Exp

    with (
        tc.tile_pool(name="const", bufs=1) as constp,
        tc.tile_pool(name="kv", bufs=1) as kv,
    ):
        ones = constp.tile([128, 1], DT_BF, tag="ones", name="ones")
        nc.vector.memset(ones, 1.0)

        # x stays resident through attention: it doubles as the K operand
        # of the scores matmul (the A-trick) and feeds the V projection.
        # xs_t[st][:, 128i:128(i+1)] = x.T[128i:128(i+1), 128st:128(st+1)]
        xs_t = [kv.tile([128, D], DT_BF, tag=f"xs{st}", name=f"xs{st}")
                for st in range(NST)]
        v_t = [kv.tile([128, D], DT_BF, tag=f"v{st}", name=f"v{st}")
               for st in range(NST)]
        tT_t = [kv.tile([128, 2 * CH], DT_BF, tag=f"tT{i}", name=f"tT{i}")
                for i in range(DT8)]

        with (
            tc.tile_pool(name="xw", bufs=1) as xw,
            tc.tile_pool(name="proj_ps", bufs=2, space="PSUM") as pps,
        ):
            # wv_t[oc][:, 512i:512(i+1)] = Wv.T[128i:128(i+1), 512oc:...]
            wv_t = [xw.tile([128, DT8 * CH], DT_BF, tag=f"wv{oc}", name=f"wv{oc}")
                    for oc in range(D // CH)]
            # a_t[:, 1024i+c] = A'[128i+p, c];  xq_t[:, 1024i+c] = xTq[128i+p, c]
            a_t = xw.tile([128, DT8 * D], DT_BF, tag="a", name="a")
            xq_t = xw.tile([128, DT8 * D], DT_BF, tag="xq", name="xq")

            # Deadline-ordered loads: first group's gate (wv[0] + xs[0],
            # ~1.25MB) split across all three rings ahead of everything
            # else; xs streams at v-projection pace; wv[1] before the oc=1
            # pass (~35us); a/xq before the t-projection (~60us).
            # wv[0] whole on scalar, xs evens on sync / odds on gpsimd:
            # pre-buffers ~1 group of lead by first-MM time so the PE never
            # starves mid-stream (measured better than "earliest first MM"
            # orderings, which trade the head wait for repeated stalls).
            nc.scalar.dma_start(out=wv_t[0], in_=wv[0])
            for st in range(NST):
                eng = nc.sync if st % 2 == 0 else nc.gpsimd
                eng.dma_start(out=xs_t[st], in_=xs[st])
            nc.scalar.dma_start(out=wv_t[1], in_=wv[1])
            nc.scalar.dma_start(out=a_t[:, 0 : DT8 * D // 2], in_=aT[0])
            nc.sync.dma_start(out=a_t[:, DT8 * D // 2 :], in_=aT[1])
            nc.gpsimd.dma_start(out=xq_t[:, 0 : DT8 * D // 2], in_=xq[0])
            nc.sync.dma_start(out=xq_t[:, DT8 * D // 2 :], in_=xq[1])

            # v projection, oc-outer so the first 16 groups need only wv[0]:
            # v[s,o] += x[i,s].T @ WvT[i,o]
            for oc in range(D // CH):
                for st in range(NST):
                    ps = pps.tile([128, CH], DT_F32, tag="pps", name="pps")
                    for i in range(DT8):
                        nc.tensor.matmul(
                            ps,
                            lhsT=xs_t[st][:, 128 * i : 128 * (i + 1)],
                            rhs=wv_t[oc][:, CH * i : CH * (i + 1)],
                            start=(i == 0),
                            stop=(i == DT8 - 1),
                        )
                    nc.scalar.copy(out=v_t[st][:, CH * oc : CH * (oc + 1)], in_=ps)
            # tT projection (score precursor; A' = Wq^T Wk / 32 on host):
            # tT[o,s] = A'[i,o].T @ xTq[i,s]
            for ot in range(DT8):
                for qc in range(2):
                    ps = pps.tile([128, CH], DT_F32, tag="pps", name="pps")
                    for i in range(DT8):
                        nc.tensor.matmul(
                            ps,
                            lhsT=a_t[:, D * i + 128 * ot : D * i + 128 * (ot + 1)],
                            rhs=xq_t[:, D * i + CH * qc : D * i + CH * (qc + 1)],
                            start=(i == 0),
                            stop=(i == DT8 - 1),
                        )
                    nc.scalar.copy(out=tT_t[ot][:, CH * qc : CH * (qc + 1)], in_=ps)

        # ---- attention ----
        with (
            tc.tile_pool(name="attn_sb", bufs=1) as asb,
            tc.tile_pool(name="mask_sb", bufs=4) as msb,
            tc.tile_pool(name="outs_sb", bufs=2) as osb,
            tc.tile_pool(name="score_ps", bufs=2, space="PSUM") as sps,
            tc.tile_pool(name="out_ps", bufs=2, space="PSUM") as ops,
            tc.tile_pool(name="den_ps", bufs=2, space="PSUM") as dps,
        ):
            p_t = {}
            for L in range(4):
                for b in range(SCHED[L]):
                    ps = sps.tile([128, QC], DT_F32, tag="sps", name="sps")
                    for i in range(DT8):
                        nc.tensor.matmul(
                            ps,
                            lhsT=xs_t[b][:, 128 * i : 128 * (i + 1)],
                            rhs=tT_t[i][:, QC * L : QC * (L + 1)],
                            start=(i == 0),
                            stop=(i == DT8 - 1),
                        )
                    m = msb.tile([128, QC], DT_BF, tag="mask", name="mask")
                    nc.sync.dma_start(out=m, in_=msk[MASK_BASE[L] + b])
                    es = msb.tile([128, QC], DT_BF, tag="es", name="es")
                    nc.scalar.activation(es, ps, Exp)
                    p = asb.tile([128, QC], DT_BF, tag=f"p{L}_{b}", name=f"p{L}_{b}")
                    nc.vector.tensor_mul(p, es, m)
                    p_t[(L, b)] = p

            for L in range(4):
                nblk = SCHED[L]
                for sqt in range(QC // 128):
                    # den first within each block, and the output halves in
                    # separate PSUM tiles: the reciprocal and the first
                    # half's scale+store start before the last matmuls of
                    # the second half finish, shortening the exposed tail.
                    po0 = ops.tile([128, CH], DT_F32, tag="po0", name="po0")
                    po1 = ops.tile([128, CH], DT_F32, tag="po1", name="po1")
                    pd = dps.tile([128, 1], DT_F32, tag="pd", name="pd")
                    for b in range(nblk):
                        pt = p_t[(L, b)][:, 128 * sqt : 128 * (sqt + 1)]
                        nc.tensor.matmul(
                            pd, lhsT=pt, rhs=ones,
                            start=(b == 0), stop=(b == nblk - 1),
                            skip_group_check=True,
                        )
                        nc.tensor.matmul(
                            po0, lhsT=pt, rhs=v_t[b][:, 0:CH],
                            start=(b == 0), stop=(b == nblk - 1),
                            skip_group_check=True,
                        )
                        nc.tensor.matmul(
                            po1, lhsT=pt, rhs=v_t[b][:, CH:D],
                            start=(b == 0), stop=(b == nblk - 1),
                            skip_group_check=True,
                        )
                    r = osb.tile([128, 1], DT_F32, tag="r", name="r")
                    nc.vector.reciprocal(r, pd)
                    o = osb.tile([128, D], DT_BF, tag="osb", name="osb")
                    row = QC * L + 128 * sqt
                    nc.vector.tensor_scalar_mul(o[:, 0:CH], po0, r)
                    nc.sync.dma_start(
                        out=out[row : row + 128, 0:CH], in_=o[:, 0:CH])
                    nc.vector.tensor_scalar_mul(o[:, CH:D], po1, r)
                    nc.scalar.dma_start(
                        out=out[row : row + 128, CH:D], in_=o[:, CH:D])


def build_program():
    nc = bacc.Bacc(
        "TRN2",
        target_bir_lowering=False,
        debug=False,
        enable_asserts=False,
        num_devices=N_CORES,
    )
    xs = nc.dram_tensor("xs", [NST, 128, D], DT_BF, kind="ExternalInput").ap()
    xq = nc.dram_tensor("xq", [2, 128, DT8 * D // 2], DT_BF, kind="ExternalInput").ap()
    aT = nc.dram_tensor("aT", [2, 128, DT8 * D // 2], DT_BF, kind="ExternalInput").ap()
    wv = nc.dram_tensor("wv", [D // CH, 128, DT8 * CH], DT_BF, kind="ExternalInput").ap()
    msk = nc.dram_tensor("msk", [NMASK, BLK, QC], DT_BF, kind="ExternalInput").ap()
    out = nc.dram_tensor("out", [2 * CH, D], DT_BF, kind="ExternalOutput").ap()
    with tile.TileContext(nc) as tc:
        _emit(tc, xs, xq, aT, wv, msk, out)
    nc.compile()
    return nc


def get_program():
    if "nc" not in _NC_CACHE:
        _NC_CACHE["nc"] = build_program()
    return _NC_CACHE["nc"]


def _chunks_for(core):
    """Per-core 256-wide query chunks, L-ordered to match SCHED=(4,8,12,16).
    Real causal k-block need: chunk j -> 2(j+1)."""
    return [0, 3, 4, 7] if core % 2 == 0 else [1, 2, 5, 6]


def _build_masks(chunks):
    """[40,128,256] in {0,1}: allowed(sk=128*blk+p, sq=256*j+c) = sk <= sq.
    Padding blocks beyond a chunk's real causal depth come out all-zero."""
    m = np.zeros((NMASK, BLK, QC), np.float32)
    p = np.arange(BLK)[:, None]
    c = np.arange(QC)[None, :]
    for L, j in enumerate(chunks):
        for b in range(SCHED[L]):
            m[MASK_BASE[L] + b] = BLK * b + p <= QC * j + c
    return m.astype(bf16)


def _pack_pi(mat, free):
    """[128*8, ncol] -> [2, 128, 4*ncol]: col block i carries rows 128i.."""
    r = mat.reshape(DT8, 128, -1).transpose(1, 0, 2).reshape(128, -1)
    return np.ascontiguousarray(r.reshape(128, 2, free).transpose(1, 0, 2))


def build_in_maps(x, Wq, Wk, Wv):
    # A' = Wq^T Wk / 32 (exact in fp32 on host): scores = x_q A' x_k^T.
    a = (Wq.T.astype(np.float32) @ Wk.astype(np.float32)) / 32.0
    a8 = _pack_pi(a.astype(bf16), DT8 * D // 2)
    # wv[oc][p][512i+c] = Wv[512oc+c, 128i+p]
    wv8 = np.ascontiguousarray(
        Wv.astype(bf16).reshape(D // CH, CH, DT8, 128).transpose(0, 3, 2, 1)
        .reshape(D // CH, 128, DT8 * CH))
    masks = {par: _build_masks(_chunks_for(par)) for par in (0, 1)}
    in_maps = []
    for core in range(N_CORES):
        b = core // 2
        chunks = _chunks_for(core)
        xb = x[b].astype(bf16)  # [S, D]
        # xs[st][p][128i+c] = x[128st+c, 128i+p]
        xs = np.ascontiguousarray(
            xb.reshape(NST, BLK, DT8, 128).transpose(0, 3, 2, 1)
            .reshape(NST, 128, D))
        xqc = np.concatenate(
            [xb[QC * j : QC * (j + 1)].T for j in chunks], axis=1)  # [D, 1024]
        in_maps.append(
            {"xs": xs, "xq": _pack_pi(xqc, DT8 * D // 2), "aT": a8,
             "wv": wv8, "msk": masks[core % 2]}
        )
    return in_maps


def assemble_output(results):
    out = np.zeros((B, S, D), np.float32)
    for core in range(N_CORES):
        b = core // 2
        for L, j in enumerate(_chunks_for(core)):
            out[b, QC * j : QC * (j + 1)] = (
                results[core]["out"][QC * L : QC * (L + 1)].astype(np.float32)
            )
    return out


def kernel(x, Wq, Wk, Wv):
    x = np.asarray(x, np.float32)
    nc = get_program()
    in_maps = build_in_maps(x, np.asarray(Wq, np.float32),
                            np.asarray(Wk, np.float32), np.asarray(Wv, np.float32))
    res = run_bass_kernel_spmd(nc, in_maps, core_ids=list(range(N_CORES)))
    return assemble_output(res.results)


# revision 13
# speedup vs baseline: 1.0674x; 1.0014x over previous
"""Causal single-head attention (B=4, S=2048, D=1024) on 8 Trainium2 NeuronCores.

Sharding: core = (batch, parity). Each batch's 4 query-chunks of 512 are split
{0,3} / {1,2} across its two cores so causal work balances exactly.

Key trick: scores = q@k.T = x_q (Wq^T Wk) x_k^T. The host precomputes
A = Wq^T Wk / 32 exactly in fp32, so the device only needs ONE projection
(t = x_q A, query-sharded) for the score path instead of replicated Q and K
projections — the scores matmul contracts t directly against the raw x
tiles already resident in SBUF. This removes the entire K projection
(~55us of PE time per core).

All inputs are host-retiled so that every SBUF tile loads with one large
contiguous DMA ([p, i*free] packed layout: column block i of a tile carries
contraction rows 128i..128(i+1)); loads are spread over the three DGE rings
(sync + scalar HWDGE, gpsimd SWDGE) in deadline order: the first
v-projection group's operands (wv half-tiles + xs[0]) lead on all three
rings, the rest of xs streams at its consumption pace, and a/xq (needed
only when the t-projection starts ~60us in) trail.

Device algorithm per core (all matmuls bf16 operands, fp32 PSUM accumulation):
  v[s,o]  = x.T @ WvT
  tT[o,s] = A'.T @ xTq          (A' = Wq^T Wk / 32, host-exact)
  scores_T[sk,sq] = x_blk.T @ tT_chunk   -> PSUM
  p = exp(scores_T) * mask   (no max-subtraction: logits are O(1) by
                              construction; masked entries multiply to 0)
  out[sq,o] = sum_blk p_blk.T @ v_blk ; den[sq] = sum_blk p_blk.T @ ones
  out = out * (1/den)        (stored bf16 to halve output DMA)
"""

import sys

if "/opt/trn_rl_repo" not in sys.path:
    sys.path.insert(0, "/opt/trn_rl_repo")

import numpy as np
import ml_dtypes

import concourse.mybir as mybir
import concourse.tile as tile
from concourse import bacc
from concourse.bass_utils import run_bass_kernel_spmd

bf16 = ml_dtypes.bfloat16

B, S, D = 4, 2048, 1024
CH = 512            # projection column-chunk width
QC = 256            # attention query-chunk width
BLK = 128           # key-block
NST = S // BLK      # 16 sequence tiles
# Per-core schedule: 4 query-chunks of 256, processed with a fixed padded
# k-block count (4,8,12,16). Host assigns real chunks sorted by causal depth
# so padding waste is exactly 4 blocks/core; masks (data) encode reality.
SCHED = (4, 8, 12, 16)
MASK_BASE = (0, 4, 12, 24)
NMASK = sum(SCHED)  # 40
DT8 = D // 128      # contraction tiles
N_CORES = 8
DT_BF = mybir.dt.bfloat16
DT_F32 = mybir.dt.float32

_NC_CACHE = {}


def _emit(tc, xs, xq, aT, wv, msk, out):
    nc = tc.nc
    Exp = mybir.ActivationFunctionType.Exp

    with (
        tc.tile_pool(name="const", bufs=1) as constp,
        tc.tile_pool(name="kv", bufs=1) as kv,
    ):
        ones = constp.tile([128, 1], DT_BF, tag="ones", name="ones")
        nc.vector.memset(ones, 1.0)

        # x stays resident through attention: it doubles as the K operand
        # of the scores matmul (the A-trick) and feeds the V projection.
        # xs_t[st][:, 128i:128(i+1)] = x.T[128i:128(i+1), 128st:128(st+1)]
        xs_t = [kv.tile([128, D], DT_BF, tag=f"xs{st}", name=f"xs{st}")
                for st in range(NST)]
        v_t = [kv.tile([128, D], DT_BF, tag=f"v{st}", name=f"v{st}")
               for st in range(NST)]
        tT_t = [kv.tile([128, 2 * CH], DT_BF, tag=f"tT{i}", name=f"tT{i}")
                for i in range(DT8)]

        with (
            tc.tile_pool(name="xw", bufs=1) as xw,
            tc.tile_pool(name="proj_ps", bufs=2, space="PSUM") as pps,
        ):
            # wv_t[oc][:, 512i:512(i+1)] = Wv.T[128i:128(i+1), 512oc:...]
            wv_t = [xw.tile([128, DT8 * CH], DT_BF, tag=f"wv{oc}", name=f"wv{oc}")
                    for oc in range(D // CH)]
            # a_t[:, 1024i+c] = A'[128i+p, c];  xq_t[:, 1024i+c] = xTq[128i+p, c]
            a_t = xw.tile([128, DT8 * D], DT_BF, tag="a", name="a")
            xq_t = xw.tile([128, DT8 * D], DT_BF, tag="xq", name="xq")

            # Deadline-ordered loads: first group's gate (wv[0] + xs[0],
            # ~1.25MB) split across all three rings ahead of everything
            # else; xs streams at v-projection pace; wv[1] before the oc=1
            # pass (~35us); a/xq before the t-projection (~60us).
            # wv[0] whole on scalar, xs evens on sync / odds on gpsimd:
            # pre-buffers ~1 group of lead by first-MM time so the PE never
            # starves mid-stream (measured better than "earliest first MM"
            # orderings, which trade the head wait for repeated stalls).
            nc.scalar.dma_start(out=wv_t[0], in_=wv[0])
            for st in range(NST):
                eng = nc.sync if st % 2 == 0 else nc.gpsimd
                eng.dma_start(out=xs_t[st], in_=xs[st])
            nc.scalar.dma_start(out=wv_t[1], in_=wv[1])
            nc.scalar.dma_start(out=a_t[:, 0 : DT8 * D // 2], in_=aT[0])
            nc.sync.dma_start(out=a_t[:, DT8 * D // 2 :], in_=aT[1])
            nc.gpsimd.dma_start(out=xq_t[:, 0 : DT8 * D // 2], in_=xq[0])
            nc.sync.dma_start(out=xq_t[:, DT8 * D // 2 :], in_=xq[1])

            # v projection, oc-outer so the first 16 groups need only wv[0]:
            # v[s,o] += x[i,s].T @ WvT[i,o]
            for oc in range(D // CH):
                for st in range(NST):
                    ps = pps.tile([128, CH], DT_F32, tag="pps", name="pps")
                    for i in range(DT8):
                        nc.tensor.matmul(
                            ps,
                            lhsT=xs_t[st][:, 128 * i : 128 * (i + 1)],
                            rhs=wv_t[oc][:, CH * i : CH * (i + 1)],
                            start=(i == 0),
                            stop=(i == DT8 - 1),
                        )
                    nc.scalar.copy(out=v_t[st][:, CH * oc : CH * (oc + 1)], in_=ps)
            # tT projection (score precursor; A' = Wq^T Wk / 32 on host):
            # tT[o,s] = A'[i,o].T @ xTq[i,s]
            for ot in range(DT8):
                for qc in range(2):
                    ps = pps.tile([128, CH], DT_F32, tag="pps", name="pps")
                    for i in range(DT8):
                        nc.tensor.matmul(
                            ps,
                            lhsT=a_t[:, D * i + 128 * ot : D * i + 128 * (ot + 1)],
                            rhs=xq_t[:, D * i + CH * qc : D * i + CH * (qc + 1)],
                            start=(i == 0),
                            stop=(i == DT8 - 1),
                        )
                    nc.scalar.copy(out=tT_t[ot][:, CH * qc : CH * (qc + 1)], in_=ps)

        # ---- attention ----
        with (
            tc.tile_pool(name="attn_sb", bufs=1) as asb,
            tc.tile_pool(name="mask_sb", bufs=4) as msb,
            tc.tile_pool(name="outs_sb", bufs=2) as osb,
            tc.tile_pool(name="score_ps", bufs=2, space="PSUM") as sps,
            tc.tile_pool(name="out_ps", bufs=2, space="PSUM") as ops,
            tc.tile_pool(name="den_ps", bufs=2, space="PSUM") as dps,
        ):
            p_t = {}
            for L in range(4):
                for b in range(SCHED[L]):
                    ps = sps.tile([128, QC], DT_F32, tag="sps", name="sps")
                    for i in range(DT8):
                        nc.tensor.matmul(
                            ps,
                            lhsT=xs_t[b][:, 128 * i : 128 * (i + 1)],
                            rhs=tT_t[i][:, QC * L : QC * (L + 1)],
                            start=(i == 0),
                            stop=(i == DT8 - 1),
                        )
                    m = msb.tile([128, QC], DT_BF, tag="mask", name="mask")
                    nc.sync.dma_start(out=m, in_=msk[MASK_BASE[L] + b])
                    es = msb.tile([128, QC], DT_BF, tag="es", name="es")
                    nc.scalar.activation(es, ps, Exp)
                    p = asb.tile([128, QC], DT_BF, tag=f"p{L}_{b}", name=f"p{L}_{b}")
                    nc.vector.tensor_mul(p, es, m)
                    p_t[(L, b)] = p

            # Largest group first: the tail after the very last matmul then
            # belongs to the 4-block L=0 group, and the big groups' output
            # DMAs overlap remaining attnV compute.
            for L in (3, 2, 1, 0):
                nblk = SCHED[L]
                for sqt in range(QC // 128):
                    # den first within each block, and the output halves in
                    # separate PSUM tiles: the reciprocal and the first
                    # half's scale+store start before the last matmuls of
                    # the second half finish, shortening the exposed tail.
                    po0 = ops.tile([128, CH], DT_F32, tag="po0", name="po0")
                    po1 = ops.tile([128, CH], DT_F32, tag="po1", name="po1")
                    pd = dps.tile([128, 1], DT_F32, tag="pd", name="pd")
                    for b in range(nblk):
                        pt = p_t[(L, b)][:, 128 * sqt : 128 * (sqt + 1)]
                        nc.tensor.matmul(
                            pd, lhsT=pt, rhs=ones,
                            start=(b == 0), stop=(b == nblk - 1),
                            skip_group_check=True,
                        )
                        nc.tensor.matmul(
                            po0, lhsT=pt, rhs=v_t[b][:, 0:CH],
                            start=(b == 0), stop=(b == nblk - 1),
                            skip_group_check=True,
                        )
                        nc.tensor.matmul(
                            po1, lhsT=pt, rhs=v_t[b][:, CH:D],
                            start=(b == 0), stop=(b == nblk - 1),
                            skip_group_check=True,
                        )
                    r = osb.tile([128, 1], DT_F32, tag="r", name="r")
                    nc.vector.reciprocal(r, pd)
                    o = osb.tile([128, D], DT_BF, tag="osb", name="osb")
                    row = QC * L + 128 * sqt
                    nc.vector.tensor_scalar_mul(o[:, 0:CH], po0, r)
                    nc.sync.dma_start(
                        out=out[row : row + 128, 0:CH], in_=o[:, 0:CH])
                    nc.vector.tensor_scalar_mul(o[:, CH:D], po1, r)
                    nc.scalar.dma_start(
                        out=out[row : row + 128, CH:D], in_=o[:, CH:D])


def build_program():
    nc = bacc.Bacc(
        "TRN2",
        target_bir_lowering=False,
        debug=False,
        enable_asserts=False,
        num_devices=N_CORES,
    )
    xs = nc.dram_tensor("xs", [NST, 128, D], DT_BF, kind="ExternalInput").ap()
    xq = nc.dram_tensor("xq", [2, 128, DT8 * D // 2], DT_BF, kind="ExternalInput").ap()
    aT = nc.dram_tensor("aT", [2, 128, DT8 * D // 2], DT_BF, kind="ExternalInput").ap()
    wv = nc.dram_tensor("wv", [D // CH, 128, DT8 * CH], DT_BF, kind="ExternalInput").ap()
    msk = nc.dram_tensor("msk", [NMASK, BLK, QC], DT_BF, kind="ExternalInput").ap()
    out = nc.dram_tensor("out", [2 * CH, D], DT_BF, kind="ExternalOutput").ap()
    with tile.TileContext(nc) as tc:
        _emit(tc, xs, xq, aT, wv, msk, out)
    nc.compile()
    return nc


def get_program():
    if "nc" not in _NC_CACHE:
        _NC_CACHE["nc"] = build_program()
    return _NC_CACHE["nc"]


def _chunks_for(core):
    """Per-core 256-wide query chunks, L-ordered to match SCHED=(4,8,12,16).
    Real causal k-block need: chunk j -> 2(j+1)."""
    return [0, 3, 4, 7] if core % 2 == 0 else [1, 2, 5, 6]


def _build_masks(chunks):
    """[40,128,256] in {0,1}: allowed(sk=128*blk+p, sq=256*j+c) = sk <= sq.
    Padding blocks beyond a chunk's real causal depth come out all-zero."""
    m = np.zeros((NMASK, BLK, QC), np.float32)
    p = np.arange(BLK)[:, None]
    c = np.arange(QC)[None, :]
    for L, j in enumerate(chunks):
        for b in range(SCHED[L]):
            m[MASK_BASE[L] + b] = BLK * b + p <= QC * j + c
    return m.astype(bf16)


def _pack_pi(mat, free):
    """[128*8, ncol] -> [2, 128, 4*ncol]: col block i carries rows 128i.."""
    r = mat.reshape(DT8, 128, -1).transpose(1, 0, 2).reshape(128, -1)
    return np.ascontiguousarray(r.reshape(128, 2, free).transpose(1, 0, 2))


def build_in_maps(x, Wq, Wk, Wv):
    # A' = Wq^T Wk / 32 (exact in fp32 on host): scores = x_q A' x_k^T.
    a = (Wq.T.astype(np.float32) @ Wk.astype(np.float32)) / 32.0
    a8 = _pack_pi(a.astype(bf16), DT8 * D // 2)
    # wv[oc][p][512i+c] = Wv[512oc+c, 128i+p]
    wv8 = np.ascontiguousarray(
        Wv.astype(bf16).reshape(D // CH, CH, DT8, 128).transpose(0, 3, 2, 1)
        .reshape(D // CH, 128, DT8 * CH))
    masks = {par: _build_masks(_chunks_for(par)) for par in (0, 1)}
    in_maps = []
    for core in range(N_CORES):
        b = core // 2
        chunks = _chunks_for(core)
        xb = x[b].astype(bf16)  # [S, D]
        # xs[st][p][128i+c] = x[128st+c, 128i+p]
        xs = np.ascontiguousarray(
            xb.reshape(NST, BLK, DT8, 128).transpose(0, 3, 2, 1)
            .reshape(NST, 128, D))
        xqc = np.concatenate(
            [xb[QC * j : QC * (j + 1)].T for j in chunks], axis=1)  # [D, 1024]
        in_maps.append(
            {"xs": xs, "xq": _pack_pi(xqc, DT8 * D // 2), "aT": a8,
             "wv": wv8, "msk": masks[core % 2]}
        )
    return in_maps


def assemble_output(results):
    out = np.zeros((B, S, D), np.float32)
    for core in range(N_CORES):
        b = core // 2
        for L, j in enumerate(_chunks_for(core)):
            out[b, QC * j : QC * (j + 1)] = (
                results[core]["out"][QC * L : QC * (L + 1)].astype(np.float32)
            )
    return out


def kernel(x, Wq, Wk, Wv):
    x = np.asarray(x, np.float32)
    nc = get_program()
    in_maps = build_in_maps(x, np.asarray(Wq, np.float32),
                            np.asarray(Wk, np.float32), np.asarray(Wv, np.float32))
    res = run_bass_kernel_spmd(nc, in_maps, core_ids=list(range(N_CORES)))
    return assemble_output(res.results)
